# revision 20
# baseline (speedup 1.0000x reference)
"""NeuralSDE forecasting kernel for 8x Trainium2 NeuronCores (Bass/Tile).

Data-parallel over batch B=256 across 8 cores (32 batch elems per core).
The per-core scan runs feature-major ("transposed"): state y.T lives in a
[128 partitions, 4*32] SBUF tile; column block k holds features
128k..128k+128 of the 32 local batch columns. Orientation: out = lhsT.T @
rhs with weight tiles stationary and the state as the 32-col moving
operand. No transposes anywhere.

Precision: the 255-step recurrence amplifies per-step rounding noise
~1000x, so bf16 operands fail (0.19 rel err) and fp32 matmuls are
weight-load bound (measured 7.3 ms: the fp32 Matmult reloads its 128-col
weight tile twice at ~225 ns). Instead each weight is split W = W_hi +
W_lo (both bf16) and each state operand y into y_hi + y_lo; the product
uses three bf16 matmuls (y_hi@W_hi + y_lo@W_hi + y_hi@W_lo, fp32 PSUM
accumulate) which restores ~fp32 accuracy (1.0e-3 measured end-to-end)
while loading weights via the 2x Fast-Weight-Load bf16 path. The two
W_hi products run as one N=64 matmul against packed [y_hi|y_lo]; the
W_lo product accumulates onto the lo half; a DVE add folds the halves.

sigmoid(x) = 0.5*(1+tanh(x/2)) keeps the scan on the Tanh ACT table only;
the 0.5 factors are folded into the host-prescaled dW. Biases ride an
appended ones-row of the control input (b1) or DVE bias adds (b2, bg).
"""

import os
import sys

sys.path.insert(0, "/opt/trn_rl_repo")

import numpy as np
import ml_dtypes

import concourse.bass as bass
import concourse.bacc as bacc
import concourse.mybir as mybir
import concourse.tile as tile
from concourse.bass_utils import run_bass_kernel_spmd

B, T, C, H, O = 256, 256, 32, 512, 32
OUT_TIME = 32
NCORES = 8
BL = B // NCORES  # 32 batch elements per core
NT = int(os.environ.get("BASS_NT", T - 1))  # 255 scan steps
SAVE0 = NT - OUT_TIME  # first step whose y_next lands in the output tail
KC = H // 128  # 4 feature chunks
F32 = mybir.dt.float32
BF16 = mybir.dt.bfloat16
BF = ml_dtypes.bfloat16

Tanh = mybir.ActivationFunctionType.Tanh
Relu = mybir.ActivationFunctionType.Relu
Identity = mybir.ActivationFunctionType.Identity

_BUILT = None


def _build_nc():
    nc = bacc.Bacc("TRN2", target_bir_lowering=False, debug=False)

    # --- DRAM I/O (per-core shards; weights replicated) ---
    # x-tilde for all t, feature-major (col = t*BL + b), hi/lo bf16 split
    NTP = 16 * 512 // BL  # 256 t-slots (255 used, 1 zero pad)
    d_xah = nc.dram_tensor("xall_hi", [C + 1, NTP * BL], BF16, kind="ExternalInput")
    d_xal = nc.dram_tensor("xall_lo", [C + 1, NTP * BL], BF16, kind="ExternalInput")
    d_x0 = nc.dram_tensor("x0", [C + 1, BL], F32, kind="ExternalInput")
    d_dw = nc.dram_tensor("dw", [NT, 128, KC * BL], F32, kind="ExternalInput")
    wnames = ["w1y", "w2", "wg"]
    d_w = {
        (n, p): nc.dram_tensor(f"{n}_{p}", [128, KC * H], BF16, kind="ExternalInput")
        for n in wnames
        for p in ("hi", "lo")
    }
    d_w1b = {
        p: nc.dram_tensor(f"w1b_{p}", [C + 1, H], BF16, kind="ExternalInput")
        for p in ("hi", "lo")
    }
    d_wini = nc.dram_tensor("wini", [C + 1, H], F32, kind="ExternalInput")
    d_bb = nc.dram_tensor("biasb", [128, KC * BL], F32, kind="ExternalInput")
    d_bc = nc.dram_tensor("biasc", [128, KC * BL], F32, kind="ExternalInput")
    d_wh1 = nc.dram_tensor("wh1", [128, KC * H], F32, kind="ExternalInput")
    d_wh2 = nc.dram_tensor("wh2", [128, KC * O], F32, kind="ExternalInput")
    d_bh1 = nc.dram_tensor("bh1t", [128, KC], F32, kind="ExternalInput")
    d_bh2 = nc.dram_tensor("bh2t", [O, 1], F32, kind="ExternalInput")
    d_out = nc.dram_tensor("out", [O, OUT_TIME * BL], F32, kind="ExternalOutput")
    # scratch for the precomputed control projection u_t = x~_t @ [W1x; b1]
    d_u = nc.dram_tensor("u_scr", [KC, 16, 128, 512], F32, kind="Internal")

    with tile.TileContext(nc) as tc:
        with (
            tc.tile_pool(name="const", bufs=1) as const,
            tc.tile_pool(name="xp", bufs=6) as xp,
            tc.tile_pool(name="dwp", bufs=6) as dwp,
            tc.tile_pool(name="yp", bufs=2) as yp,
            tc.tile_pool(name="tmp", bufs=3) as tmp,
            tc.tile_pool(name="pp", bufs=2, space="PSUM") as pp,
        ):
            # --- resident weights ---
            w_s = {}
            for key, d in d_w.items():
                w_s[key] = const.tile(
                    [128, KC * H], BF16, tag=f"{key[0]}_{key[1]}",
                    name=f"{key[0]}_{key[1]}_s",
                )
                nc.sync.dma_start(out=w_s[key][:], in_=d[:])
            w1b_s = {}
            for p, d in d_w1b.items():
                w1b_s[p] = const.tile([C + 1, H], BF16, tag=f"w1b{p}", name=f"w1b_{p}_s")
                nc.sync.dma_start(out=w1b_s[p][:], in_=d[:])
            wini = const.tile([C + 1, H], F32, tag="wini")
            biasb = const.tile([128, KC * BL], F32, tag="biasb")
            biasc = const.tile([128, KC * BL], F32, tag="biasc")
            wh1 = const.tile([128, KC * H], F32, tag="wh1")
            wh2 = const.tile([128, KC * O], F32, tag="wh2")
            bh1 = const.tile([128, KC], F32, tag="bh1")
            bh2 = const.tile([O, 1], F32, tag="bh2")
            x0 = const.tile([C + 1, BL], F32, tag="x0")
            slab = const.tile([128, OUT_TIME * 128], F32, tag="slab")
            rT = const.tile([128, KC * 1024], F32, tag="rT")
            outs = const.tile([O, OUT_TIME * BL], F32, tag="outs")
            for dst, src in [
                (wini, d_wini), (biasb, d_bb), (biasc, d_bc), (wh1, d_wh1),
                (wh2, d_wh2), (bh1, d_bh1), (bh2, d_bh2), (x0, d_x0),
            ]:
                nc.sync.dma_start(out=dst[:], in_=src[:])

            def wsl(n, p, k, m):  # lhsT tile (k, m) of weight n, part p
                return w_s[(n, p)][:, k * H + m * 128 : k * H + (m + 1) * 128]

            # double-bf16 matmul group. psum [128, 256]: hi-sums contiguous in
            # cols 0:128 (m-chunk m at 32m), lo-sums in cols 128:256. The
            # N=64 W_hi matmul streams [y_hi[k] | y_lo[k]] via a strided rhs
            # AP and scatters its two 32-col halves to the hi/lo blocks via a
            # strided out AP; the W_lo matmul accumulates onto the lo block.
            # Strides live on the PE APs so every DVE op stays contiguous.
            # All W_lo x y_hi products are emitted first: they depend only on
            # the hi half of the state, so the PE starts them one DVE hop
            # after tanh while the lo half is still being computed. The
            # mixed-pending first W_hi matmul relies on PSUM's per-element
            # has_written accumulate-vs-overwrite semantics.
            def mm_group(ps, wname, rhs_hl):
                pv = ps[:].rearrange("p (h q) -> p h q", h=2)
                rv = rhs_hl[:].rearrange("p (h q) -> p h q", h=2)
                for m in range(KC):
                    hilo = pv[:, :, m * BL : (m + 1) * BL]
                    lo = ps[:, 128 + m * BL : 128 + (m + 1) * BL]
                    for k in range(KC):
                        nc.tensor.matmul(
                            lo, wsl(wname, "lo", k, m),
                            rhs_hl[:, k * BL : (k + 1) * BL],
                            start=(k == 0), stop=False,
                        )
                    for k in range(KC):
                        nc.tensor.matmul(
                            hilo, wsl(wname, "hi", k, m),
                            rv[:, :, k * BL : (k + 1) * BL],
                            start=False, stop=(k == KC - 1),
                        )

            def fold(dst, ps, extra=None):
                # dst[128, 128] = hi block + lo block (+ extra); DVE may read
                # only one PSUM operand per op, so fold in two steps. All
                # operands contiguous [128, 128].
                if extra is not None:
                    nc.vector.tensor_add(dst, ps[:, 0:128], extra)
                else:
                    nc.vector.tensor_copy(dst, ps[:, 0:128])
                nc.vector.tensor_add(dst, dst, ps[:, 128:256])

            def split_hl(hl_tile, src_ap):
                # hl_tile [128, 256]: y_hi in cols 0:128, y_lo in 128:256
                nc.vector.tensor_copy(hl_tile[:, 0:128], src_ap)
                nc.vector.tensor_sub(hl_tile[:, 128:256], src_ap, hl_tile[:, 0:128])

            # --- precompute u_t = x~_t @ [W1x; b1] for all t (batched N=512,
            # 16 timesteps per matmul) into DRAM scratch ---
            xall_hi = const.tile([C + 1, NTP * BL], BF16, tag="xallhi")
            xall_lo = const.tile([C + 1, NTP * BL], BF16, tag="xalllo")
            nc.sync.dma_start(out=xall_hi[:], in_=d_xah[:])
            nc.sync.dma_start(out=xall_lo[:], in_=d_xal[:])
            for m in range(KC):
                for cch in range(16):
                    psU = pp.tile([128, 512], F32, tag="psB", name=f"psU_{m}_{cch}")
                    cs = slice(cch * 512, (cch + 1) * 512)
                    wbh = w1b_s["hi"][:, m * 128 : (m + 1) * 128]
                    wbl = w1b_s["lo"][:, m * 128 : (m + 1) * 128]
                    nc.tensor.matmul(psU[:], wbh, xall_hi[:, cs], start=True, stop=False)
                    nc.tensor.matmul(psU[:], wbh, xall_lo[:, cs], start=False, stop=False)
                    nc.tensor.matmul(psU[:], wbl, xall_hi[:, cs], start=False, stop=True)
                    usb = tmp.tile([128, 512], F32, tag="usb", name=f"usb_{m}_{cch}")
                    nc.vector.tensor_copy(usb[:], psU[:])
                    nc.sync.dma_start(out=d_u[m, cch], in_=usb[:])

            # --- z0 (fp32, one-off) ---
            ps0 = pp.tile([128, 2 * 128], F32, tag="psA")
            for m in range(KC):
                nc.tensor.matmul(
                    ps0[:, m * BL : (m + 1) * BL],
                    wini[:, m * 128 : (m + 1) * 128], x0[:],
                    start=True, stop=True,
                )
            y_t = yp.tile([128, KC * BL], F32, tag="y")
            nc.vector.tensor_copy(y_t[:], ps0[:, 0:128])
            y = y_t[:]
            yhl = tmp.tile([128, KC * 2 * BL], BF16, tag="yhl", name="yhl_init")
            split_hl(yhl, y)

            # --- scan ---
            for t in range(NT):
                u_t = xp.tile([128, KC * BL], F32, tag="u", name=f"u_{t}")
                nc.sync.dma_start(
                    out=u_t[:].rearrange("p (m b) -> p m b", m=KC),
                    in_=d_u[:, t // 16, :, (t % 16) * BL : (t % 16 + 1) * BL]
                    .rearrange("m p b -> p m b"),
                )
                dw_t = dwp.tile([128, KC * BL], F32, tag="dw", name=f"dw_{t}")
                nc.sync.dma_start(out=dw_t[:], in_=d_dw[t])

                # h = tanh(y@W1y + u)
                psA = pp.tile([128, 2 * 128], F32, tag="psA", name=f"psA_{t}")
                mm_group(psA, "w1y", yhl)
                preA = tmp.tile([128, KC * BL], F32, tag="preA", name=f"preA_{t}")
                fold(preA[:], psA, extra=u_t[:])
                # tanh evaluated twice on ACT: the bf16-output instance IS
                # h_hi (spline is deterministic, so bf16(tanh(x)) here equals
                # rounding the f32 result), letting the next matmul group
                # start one ACT op after the fold with no DVE hop; the f32
                # instance and the h_lo subtract run off the critical path.
                hhl = tmp.tile([128, KC * 2 * BL], BF16, tag="hhl", name=f"hhl_{t}")
                nc.scalar.activation(hhl[:, 0:128], preA[:], Tanh)
                h = tmp.tile([128, KC * BL], F32, tag="h", name=f"h_{t}")
                nc.scalar.activation(h[:], preA[:], Tanh)
                nc.vector.tensor_sub(hhl[:, 128:256], h[:], hhl[:, 0:128])

                # tau = tanh((y@Wg + bg)/2)  (sigmoid fold)
                psC = pp.tile([128, 2 * 128], F32, tag="psC", name=f"psC_{t}")
                mm_group(psC, "wg", yhl)
                preC = tmp.tile([128, KC * BL], F32, tag="preC", name=f"preC_{t}")
                fold(preC[:], psC, extra=biasc[:])
                tau = tmp.tile([128, KC * BL], F32, tag="tau", name=f"tau_{t}")
                nc.scalar.activation(tau[:], preC[:], Tanh, scale=0.5)
                # t1 = (tau + 1) * dw ;  dw pre-scaled by 0.5*sqrt(dt)/dt
                t1 = tmp.tile([128, KC * BL], F32, tag="t1", name=f"t1_{t}")
                nc.vector.scalar_tensor_tensor(
                    t1[:], tau[:], 1.0, dw_t[:],
                    mybir.AluOpType.add, mybir.AluOpType.mult,
                )
                yh2 = tmp.tile([128, KC * BL], F32, tag="yh2", name=f"yh2_{t}")
                nc.vector.tensor_add(yh2[:], y, t1[:])

                # f = tanh(h@W2 + b2)
                psB = pp.tile([128, 2 * 128], F32, tag="psB", name=f"psB_{t}")
                mm_group(psB, "w2", hhl)
                preB = tmp.tile([128, KC * BL], F32, tag="preB", name=f"preB_{t}")
                fold(preB[:], psB, extra=biasb[:])
                f = tmp.tile([128, KC * BL], F32, tag="f", name=f"f_{t}")
                nc.scalar.activation(f[:], preB[:], Tanh)

                # y_next = (y + t1) + f ; tail states land in the slab.
                # y_hi is produced first (bf16 add) so next-step matmuls that
                # need only the hi half start one DVE op after tanh; the f32
                # master and the lo residual follow off the critical path.
                if t >= SAVE0:
                    y2 = slab[:, (t - SAVE0) * 128 : (t - SAVE0 + 1) * 128]
                else:
                    y2_t = yp.tile([128, KC * BL], F32, tag="y", name=f"y_{t}")
                    y2 = y2_t[:]
                yhl = tmp.tile([128, KC * 2 * BL], BF16, tag="yhl", name=f"yhl_{t}")
                nc.vector.tensor_add(yhl[:, 0:128], yh2[:], f[:])
                nc.vector.tensor_add(y2, yh2[:], f[:])
                nc.vector.tensor_sub(yhl[:, 128:256], y2, yhl[:, 0:128])
                y = y2

            # --- head (fp32): out = relu(z_tail@Wh1 + bh1) @ Wh2 + bh2 ---
            # slab columns: s*128 + k*32 + b  (s = tail step, k = feat chunk)
            slab_r = slab[:].rearrange(
                "p (s k b) -> p s k b", s=OUT_TIME, k=KC, b=BL
            )
            for m in range(KC):
                for hf in range(2):
                    ps1 = pp.tile([128, 512], F32, tag="psA", name=f"ps1_{m}_{hf}")
                    for k in range(KC):
                        nc.tensor.matmul(
                            ps1[:],
                            wh1[:, k * H + m * 128 : k * H + (m + 1) * 128],
                            slab_r[:, hf * 16 : (hf + 1) * 16, k, :],
                            start=(k == 0), stop=(k == KC - 1),
                        )
                    nc.scalar.activation(
                        rT[:, m * 1024 + hf * 512 : m * 1024 + (hf + 1) * 512],
                        ps1[:], Relu, bias=bh1[:, m : m + 1],
                    )
            for hf in range(2):
                ps2 = pp.tile([O, 512], F32, tag="psB", name=f"ps2_{hf}")
                for m in range(KC):
                    nc.tensor.matmul(
                        ps2[:],
                        wh2[:, m * O : (m + 1) * O],
                        rT[:, m * 1024 + hf * 512 : m * 1024 + (hf + 1) * 512],
                        start=(m == 0), stop=(m == KC - 1),
                    )
                nc.scalar.activation(
                    outs[:, hf * 512 : (hf + 1) * 512], ps2[:], Identity,
                    bias=bh2[:],
                )
            nc.sync.dma_start(out=d_out[:], in_=outs[:])

    nc.compile()
    return nc


def _split(w):
    hi = np.asarray(w, BF)
    lo = (np.asarray(w, np.float32) - hi.astype(np.float32)).astype(BF)
    return hi, lo


def _prep_inputs(times, coeffs, final_index, dW, W_init, b_init, W1, b1, W2,
                 b2, Wg, bg, Wh1, bh1, Wh2, bh2):
    f32 = np.float32
    times = np.asarray(times, f32)
    dt = f32(max(np.min(times[1:] - times[:-1]), f32(0.001)))
    sq = f32(np.sqrt(dt))

    def lhsT_layout(w):  # [H, H] -> [128, KC*H] with (k,m) tile at k*H+m*128
        return np.ascontiguousarray(
            np.asarray(w, f32).reshape(KC, 128, H).transpose(1, 0, 2).reshape(128, KC * H)
        )

    def bias_bcast(b):  # [H] -> [128, KC*BL] feature-major broadcast
        return np.ascontiguousarray(
            np.broadcast_to(
                np.asarray(b, f32).reshape(KC, 128).T[:, :, None], (128, KC, BL)
            ).reshape(128, KC * BL)
        )

    W1 = np.asarray(W1, f32)
    shared = {}
    for name, w in [("w1y", dt * W1[:H]), ("w2", np.asarray(W2, f32)),
                    ("wg", dt * np.asarray(Wg, f32))]:
        hi, lo = _split(lhsT_layout(w))
        shared[f"{name}_hi"] = hi
        shared[f"{name}_lo"] = lo
    w1b = np.vstack([W1[H:], np.asarray(b1, f32)[None, :]])
    shared["w1b_hi"], shared["w1b_lo"] = _split(w1b)
    shared["wini"] = np.ascontiguousarray(
        np.vstack([np.asarray(W_init, f32), np.asarray(b_init, f32)[None, :]]) / dt
    )
    shared["biasb"] = bias_bcast(b2)
    shared["biasc"] = bias_bcast(bg)
    shared["wh1"] = lhsT_layout(dt * np.asarray(Wh1, f32))
    shared["wh2"] = np.ascontiguousarray(
        np.asarray(Wh2, f32).reshape(KC, 128, O).transpose(1, 0, 2).reshape(128, KC * O)
    )
    shared["bh1t"] = np.ascontiguousarray(np.asarray(bh1, f32).reshape(KC, 128).T)
    shared["bh2t"] = np.asarray(bh2, f32).reshape(O, 1)

    coeffs = np.asarray(coeffs, f32)  # [B, T, C]
    dW = np.asarray(dW, f32)  # [NT_full, B, H]
    dw_scale = f32(0.5 * sq / dt)
    in_maps = []
    NTP = 16 * 512 // BL
    for c in range(NCORES):
        bs = slice(c * BL, (c + 1) * BL)
        xt = np.empty((T, C + 1, BL), f32)
        xt[:, :C, :] = coeffs[bs].transpose(1, 2, 0)
        xt[:, C, :] = 1.0
        # all-t feature-major [C+1, t*BL+b] with zero pad to 16*512 cols
        xall = np.zeros((C + 1, NTP * BL), f32)
        xall[:, : NT * BL] = xt[:NT].transpose(1, 0, 2).reshape(C + 1, NT * BL)
        xahi, xalo = _split(xall)
        dwc = (dW[:NT, bs, :] * dw_scale).transpose(0, 2, 1)  # [NT, H, BL]
        dwc = np.ascontiguousarray(
            dwc.reshape(NT, KC, 128, BL).transpose(0, 2, 1, 3).reshape(NT, 128, KC * BL)
        )
        in_maps.append(
            {"xall_hi": xahi, "xall_lo": xalo, "x0": np.ascontiguousarray(xt[0]),
             "dw": dwc, **shared}
        )
    return in_maps


def kernel(**inputs):
    global _BUILT
    if _BUILT is None:
        _BUILT = _build_nc()
    nc = _BUILT
    in_maps = _prep_inputs(**inputs)
    res = run_bass_kernel_spmd(nc, in_maps, core_ids=list(range(NCORES)))
    out = np.empty((B, OUT_TIME, O), np.float32)
    for c, r in enumerate(res.results):
        out[c * BL : (c + 1) * BL] = (
            r["out"].reshape(O, OUT_TIME, BL).transpose(2, 1, 0)
        )
    return out
